# revision 11
# baseline (speedup 1.0000x reference)
"""Trainium2 Bass kernel for nn_Actor_87497073754359.

Math (per batch b of B=128, x[b] is [N=2048, D=128] f32):
  graph_emb = mean_n x[b];  first/curr = x[b, idx]
  q = Wq @ (W_lin @ concat(graph_emb, first, curr) + b_lin) + bq  -> [H=8, HD=16]
  scores[h, n] = q[h] . (x @ Wk.T)[n, h*16:+16] / 4 ; mask; softmax over n
  out[b] = mean_h softmax

Never materialize k = x@Wk.T. Fold q into Wk:
  t[b][c, h] = sum_j Wk[j, c] * headsel_h(j) * q[b, j] * 0.25
  scores[b][h, n] = sum_c t[b][c, h] * xT[b][c, n]
x streams once as a host-pretransposed bf16 copy, interleaved over two
DMA queues (sync: even tiles, gpsimd: odd tiles) to saturate HBM while
keeping per-quad arrival order.

Layout: all 16 batches' heads share one PSUM tile per n-chunk of 512
(row = 8*b + h -> 128 rows).  Per chunk: one mask matmul, 16 per-batch
score matmuls (zero-padded [128,32] stationaries via PE column tiling),
one Exp, one combine matmul (rmat folds 1/Z and the 1/H head-average).
Row sums for the mean are spread over DVE / ACT / PE / GPSIMD (one
batch each per quad), fully overlapped with the x DMA stream.  The last
batch streams as two half-tiles with ACT partial sums and gets its own
mini q-chain so only its final 4 matmuls + softmax trail the stream.
1/N is folded into the host-combined Wq@W_lin.

Sharding: pure data parallel over batch (16/core), no collectives.
"""

import numpy as np
import ml_dtypes

import concourse.bass as bass
import concourse.tile as tile
from concourse import bacc, mybir
from concourse.bass_utils import run_bass_kernel_spmd

B, N, D, H = 128, 2048, 128, 8
HD = D // H
NCORES = 8
BPC = B // NCORES          # 16 batches per core
P = 128
CH = 512                   # psum-bank chunk of n
NCH = N // CH              # 4
NQ = 4                     # batch quads per core
QS = BPC // NQ             # 4 batches per quad
LASTB = BPC - 1
MASKVAL = -1000.0          # exp(-1000 + s) == 0.0 exactly in f32

BF16 = mybir.dt.bfloat16
F32 = mybir.dt.float32
I32 = mybir.dt.int32


def build_kernel_body(ctx, tc):
    nc = tc.nc

    # ---- DRAM parameters (per-core shapes) ----
    xt = nc.dram_tensor("xt", [BPC, P, N], BF16, kind="ExternalInput")
    xn = nc.dram_tensor("xn", [BPC * N, D], BF16, kind="ExternalInput")
    gidx = nc.dram_tensor("gidx", [2 * BPC, 1], I32, kind="ExternalInput")
    maskneg = nc.dram_tensor("maskneg", [BPC, N], BF16, kind="ExternalInput")
    indmask = nc.dram_tensor("indmask", [P, P], BF16, kind="ExternalInput")
    ident32_d = nc.dram_tensor("ident32", [2 * BPC, 2 * BPC], BF16, kind="ExternalInput")
    ident128_d = nc.dram_tensor("ident128", [P, P], BF16, kind="ExternalInput")
    ind16 = nc.dram_tensor("ind16", [P, BPC], BF16, kind="ExternalInput")
    wcombt = nc.dram_tensor("wcombt", [3, P, D], BF16, kind="ExternalInput")
    wk = nc.dram_tensor("wk", [D, D], BF16, kind="ExternalInput")
    headscat = nc.dram_tensor("headscat", [D, P], BF16, kind="ExternalInput")
    biasq = nc.dram_tensor("biasq", [D, 1], F32, kind="ExternalInput")
    out = nc.dram_tensor("out", [BPC, N], F32, kind="ExternalOutput")

    consts = ctx.enter_context(tc.tile_pool(name="consts", bufs=1))
    xt_pool = ctx.enter_context(tc.tile_pool(name="xt", bufs=BPC))
    small = ctx.enter_context(tc.tile_pool(name="small", bufs=2))
    mscr_pool = ctx.enter_context(tc.tile_pool(name="mscr", bufs=2))
    gscr_pool = ctx.enter_context(tc.tile_pool(name="gscr", bufs=2))
    w_pool = ctx.enter_context(tc.tile_pool(name="w", bufs=NCH))
    psum_small = ctx.enter_context(tc.tile_pool(name="ps_small", bufs=2, space="PSUM"))
    psum_scores = ctx.enter_context(
        tc.tile_pool(name="ps_scores", bufs=NCH, space="PSUM")
    )
    psum_out = ctx.enter_context(tc.tile_pool(name="ps_out", bufs=2, space="PSUM"))

    # ---- x stream tiles ----
    xt_tiles = [
        xt_pool.tile([P, N], BF16, tag="xt", name=f"xt{b}") for b in range(BPC)
    ]
    # even tiles on the sync queue, in order
    for b in range(0, BPC, 2):
        nc.sync.dma_start(xt_tiles[b], xt[b])
    # gpsimd queue: gather index first, a couple of odd tiles, the gather
    # (needed by every q-chain), then the rest of the odd tiles; the last
    # batch arrives as two half-tiles so its row-sum partials pipeline.
    gidx_sb = consts.tile([2 * BPC, 1], I32)
    nc.gpsimd.dma_start(gidx_sb, gidx[:])
    nc.gpsimd.dma_start(xt_tiles[1], xt[1])
    nc.gpsimd.dma_start(xt_tiles[3], xt[3])
    grows = consts.tile([2 * BPC, D], BF16)
    nc.gpsimd.indirect_dma_start(
        out=grows[:],
        out_offset=None,
        in_=xn[:],
        in_offset=bass.IndirectOffsetOnAxis(ap=gidx_sb[:, :1], axis=0),
    )
    for b in range(5, LASTB, 2):
        nc.gpsimd.dma_start(xt_tiles[b], xt[b])
    nc.gpsimd.dma_start(xt_tiles[LASTB][:, : N // 2], xt[LASTB, :, : N // 2])
    nc.gpsimd.dma_start(xt_tiles[LASTB][:, N // 2 :], xt[LASTB, :, N // 2 :])

    # ---- constants into SBUF (scalar queue), in dependency-priority order ----
    maskneg_sb = consts.tile([P, N], BF16)
    nc.vector.memset(maskneg_sb, 0.0)
    nc.scalar.dma_start(maskneg_sb[:BPC, :], maskneg[:])
    indmask_sb = consts.tile([P, P], BF16)
    nc.scalar.dma_start(indmask_sb, indmask[:])
    ident32 = consts.tile([2 * BPC, 2 * BPC], BF16)
    nc.scalar.dma_start(ident32, ident32_d[:])
    ident128 = consts.tile([P, P], BF16)
    nc.scalar.dma_start(ident128, ident128_d[:])
    wcombt_sb = consts.tile([P, 3, D], BF16)
    nc.scalar.dma_start(wcombt_sb, wcombt[:].rearrange("p c j -> c p j"))
    wk_sb = consts.tile([D, D], BF16)
    nc.scalar.dma_start(wk_sb, wk[:])
    headscat_sb = consts.tile([D, NQ, 32], BF16)
    nc.scalar.dma_start(headscat_sb[:].rearrange("d q x -> d (q x)"), headscat[:])
    biasq_sb = consts.tile([D, 1], F32)
    nc.scalar.dma_start(biasq_sb, biasq[:])
    ind16_sb = consts.tile([P, BPC], BF16)
    nc.scalar.dma_start(ind16_sb, ind16[:])

    # ---- PE warm-up: ~4us of dense matmuls so HAM reaches 8/8 early ----
    warm_src = consts.tile([P, CH], BF16)
    nc.vector.memset(warm_src, 1.0)
    for i in range(6):
        pw = psum_small.tile([P, CH], F32, tag="ps", name=f"warm{i}")
        nc.tensor.matmul(
            out=pw[:], lhsT=warm_src[:, :P], rhs=warm_src[:], start=True, stop=True
        )

    # ---- the 4 score psum tiles (one per n-chunk), mask matmul first ----
    score_ps = []
    for ch in range(NCH):
        ps = psum_scores.tile([P, CH], F32, space="PSUM", tag="pscore", name=f"sc{ch}")
        nc.tensor.matmul(
            out=ps[:],
            lhsT=indmask_sb[:],
            rhs=maskneg_sb[:, ch * CH : (ch + 1) * CH],
            start=True,
            stop=False,
            skip_group_check=True,
        )
        score_ps.append(ps)

    # ---- gathered rows -> featsT [128, 32] bf16 (transpose on PE) ----
    psum_f = psum_small.tile([P, 2 * BPC], BF16, space="PSUM", tag="ps")
    nc.tensor.transpose(psum_f[:], grows[:], ident32[:])
    featsT_sb = consts.tile([P, 2 * BPC], BF16)
    nc.vector.tensor_copy(featsT_sb[:], psum_f[:])

    # ---- per quad: means (DVE/ACT/PE/GPSIMD), q-chain, score matmuls ----
    # sums_f32 col b = row-sum of batch b; col BPC = second partial of LASTB
    sums_f32 = consts.tile([P, BPC + 1], F32)
    sums_bf = consts.tile([P, BPC + 1], BF16)

    def emit_mean_dve(b, col, lo=0, hi=N):
        nc.vector.tensor_reduce(
            out=sums_f32[:, col : col + 1],
            in_=xt_tiles[b][:, lo:hi],
            axis=mybir.AxisListType.X,
            op=mybir.AluOpType.add,
        )

    def emit_mean_act(b, col, lo=0, hi=N):
        scr = mscr_pool.tile([P, N], BF16, tag="mscr", name=f"mscr{b}_{col}")
        nc.scalar.activation(
            out=scr[:, lo:hi],
            in_=xt_tiles[b][:, lo:hi],
            func=mybir.ActivationFunctionType.Copy,
            accum_out=sums_f32[:, col : col + 1],
        )

    def emit_mean_pe(b, col):
        # identity-stationary matmul: psum[:, j] accumulates x[:, k*512 + j]
        pm = psum_small.tile([P, CH], F32, space="PSUM", tag="ps", name=f"pm{b}")
        for k in range(NCH):
            nc.tensor.matmul(
                out=pm[:],
                lhsT=ident128[:],
                rhs=xt_tiles[b][:, k * CH : (k + 1) * CH],
                start=(k == 0),
                stop=(k == NCH - 1),
            )
        pescr = mscr_pool.tile([P, CH], BF16, tag="pescr", name=f"pescr{b}")
        nc.scalar.activation(
            out=pescr[:],
            in_=pm[:],
            func=mybir.ActivationFunctionType.Copy,
            accum_out=sums_f32[:, col : col + 1],
        )

    def emit_mean_gps(b, col):
        # gpsimd folds the tile in half; DVE reduces the folded half
        gscr = gscr_pool.tile([P, N // 2], BF16, tag="gscr", name=f"gscr{b}")
        nc.gpsimd.tensor_tensor(
            out=gscr[:],
            in0=xt_tiles[b][:, : N // 2],
            in1=xt_tiles[b][:, N // 2 :],
            op=mybir.AluOpType.add,
        )
        emit_mean_dve_src(gscr, col)

    def emit_mean_dve_src(src, col):
        nc.vector.tensor_reduce(
            out=sums_f32[:, col : col + 1],
            in_=src[:],
            axis=mybir.AxisListType.X,
            op=mybir.AluOpType.add,
        )

    def emit_chain(q, cols, hs_lo, hs_n, extra_partial=False, name=""):
        """q-chain for batches [cols] of quad q -> returns statq tile [P, 32*len]."""
        nbat = len(cols)
        nc.vector.tensor_copy(
            sums_bf[:, cols[0] : cols[-1] + 1], sums_f32[:, cols[0] : cols[-1] + 1]
        )
        if extra_partial:
            nc.vector.tensor_copy(sums_bf[:, BPC : BPC + 1], sums_f32[:, BPC : BPC + 1])
        psum_q = psum_small.tile([P, nbat], F32, space="PSUM", tag="ps", name=f"pq{name}")
        ctx_chunks = [
            sums_bf[:, cols[0] : cols[-1] + 1],
            featsT_sb[:, cols[0] : cols[-1] + 1],
            featsT_sb[:, BPC + cols[0] : BPC + cols[-1] + 1],
        ]
        for pch in range(3):
            nc.tensor.matmul(
                out=psum_q[:],
                lhsT=wcombt_sb[:, pch, :],
                rhs=ctx_chunks[pch],
                start=(pch == 0),
                stop=(pch == 2 and not extra_partial),
                skip_group_check=True,
            )
        if extra_partial:
            # second half-sum of the last batch folds in via one FD=1 matmul
            nc.tensor.matmul(
                out=psum_q[:, nbat - 1 : nbat],
                lhsT=wcombt_sb[:, 0, :],
                rhs=sums_bf[:, BPC : BPC + 1],
                start=False,
                stop=True,
                skip_group_check=True,
            )
        qb = small.tile([P, nbat], BF16, tag="qb", name=f"qb{name}")
        nc.vector.tensor_scalar(
            out=qb[:],
            in0=psum_q[:],
            scalar1=biasq_sb[:, 0:1],
            scalar2=None,
            op0=mybir.AluOpType.add,
        )
        # qm[j, 32s + x] = headscat[j, hs_lo+s, x] * qb[j, s]
        qm = small.tile([P, hs_n, 32], BF16, tag="qm", name=f"qm{name}")
        nc.vector.tensor_tensor(
            out=qm[:],
            in0=headscat_sb[:, hs_lo : hs_lo + hs_n, :],
            in1=qb[:, :, None].to_broadcast([P, hs_n, 32]),
            op=mybir.AluOpType.mult,
        )
        psum_t = psum_small.tile(
            [P, hs_n * 32], F32, space="PSUM", tag="ps", name=f"pt{name}"
        )
        nc.tensor.matmul(
            out=psum_t[:],
            lhsT=wk_sb[:],
            rhs=qm[:].rearrange("p q x -> p (q x)"),
            start=True,
            stop=True,
        )
        statq = consts.tile([P, hs_n * 32], BF16, name=f"statq{name}")
        nc.vector.tensor_copy(statq[:], psum_t[:])
        return statq

    def emit_scores(q, s, b, statq, stat_s, stop):
        for ch in range(NCH):
            nc.tensor.matmul(
                out=score_ps[ch][32 * q : 32 * q + 32, :],
                lhsT=statq[:, 32 * stat_s : 32 * stat_s + 32],
                rhs=xt_tiles[b][:, ch * CH : (ch + 1) * CH],
                start=False,
                stop=stop,
                skip_group_check=True,
                tile_position=(0, 32 * q),
            )

    for q in range(NQ):
        b0 = q * QS
        last_quad = q == NQ - 1
        # means: s0 -> DVE, s1 -> ACT, s2 -> PE, s3 -> GPSIMD-fold (or the
        # two-half ACT path for the very last batch)
        emit_mean_act(b0 + 1, b0 + 1)
        emit_mean_dve(b0, b0)
        emit_mean_pe(b0 + 2, b0 + 2)
        if last_quad:
            emit_mean_act(LASTB, LASTB, 0, N // 2)
            emit_mean_act(LASTB, BPC, N // 2, N)
        else:
            emit_mean_gps(b0 + 3, b0 + 3)

        if not last_quad:
            statq = emit_chain(q, list(range(b0, b0 + QS)), 0, NQ, name=f"{q}")
            for s in range(QS):
                emit_scores(q, s, b0 + s, statq, s, stop=False)
        else:
            statA = emit_chain(q, [b0, b0 + 1, b0 + 2], 0, 3, name="A")
            for s in range(3):
                emit_scores(q, s, b0 + s, statA, s, stop=False)
            statB = emit_chain(q, [LASTB], 3, 1, extra_partial=True, name="B")
            emit_scores(q, 3, LASTB, statB, 0, stop=True)

    # ---- exp (ACT), Z (DVE), rmat, combine (PE), copy out, DMA ----
    zpart = consts.tile([P, NCH], F32)
    ztot = consts.tile([P, 1], F32)
    recip = consts.tile([P, 1], F32)
    rmat = consts.tile([P, BPC], BF16)
    w_tiles = []
    for ch in range(NCH):
        wt = w_pool.tile([P, CH], BF16, tag="w", name=f"w{ch}")
        nc.scalar.activation(
            out=wt[:],
            in_=score_ps[ch][:],
            func=mybir.ActivationFunctionType.Exp,
        )
        nc.vector.tensor_reduce(
            out=zpart[:, ch : ch + 1],
            in_=wt[:],
            axis=mybir.AxisListType.X,
            op=mybir.AluOpType.add,
        )
        w_tiles.append(wt)
    nc.vector.tensor_reduce(
        out=ztot[:], in_=zpart[:], axis=mybir.AxisListType.X, op=mybir.AluOpType.add
    )
    nc.vector.reciprocal(recip[:], ztot[:])
    nc.vector.tensor_scalar(
        out=rmat[:],
        in0=ind16_sb[:],
        scalar1=recip[:, 0:1],
        scalar2=None,
        op0=mybir.AluOpType.mult,
    )
    out_sb = consts.tile([BPC, N], F32)
    for ch in range(NCH):
        psum_o = psum_out.tile([BPC, CH], F32, space="PSUM", tag="po")
        nc.tensor.matmul(
            out=psum_o[:], lhsT=rmat[:], rhs=w_tiles[ch][:], start=True, stop=True
        )
        cp = nc.scalar.copy if ch % 2 == 0 else nc.vector.tensor_copy
        cp(out_sb[:, ch * CH : (ch + 1) * CH], psum_o[:])
        nc.sync.dma_start(
            out[:, ch * CH : (ch + 1) * CH], out_sb[:, ch * CH : (ch + 1) * CH]
        )


_NC_CACHE = None


def build_nc():
    global _NC_CACHE
    if _NC_CACHE is not None:
        return _NC_CACHE
    from contextlib import ExitStack

    nc = bacc.Bacc("TRN2", target_bir_lowering=False, debug=False)
    with tile.TileContext(nc) as tc:
        with ExitStack() as ctx:
            build_kernel_body(ctx, tc)
    nc.compile()
    _NC_CACHE = nc
    return nc


def make_in_maps(x, first_node, current_node, mask, W_lin, b_lin, Wq, bq, Wk, bk):
    """Host-side sharding/layout prep. Returns list of 8 per-core input dicts."""
    x = np.asarray(x, dtype=np.float32)
    mask = np.asarray(mask)
    first_node = np.asarray(first_node).astype(np.int32)
    current_node = np.asarray(current_node).astype(np.int32)
    W_lin = np.asarray(W_lin, dtype=np.float32)
    b_lin = np.asarray(b_lin, dtype=np.float32)
    Wq = np.asarray(Wq, dtype=np.float32)
    bq_v = np.asarray(bq, dtype=np.float32)
    Wk = np.asarray(Wk, dtype=np.float32)

    xbf = x.astype(ml_dtypes.bfloat16)

    # replicated weights; 1/N for the mean is folded into Wcomb chunk 0
    wcomb = (Wq @ W_lin).astype(np.float32)            # [D, 3D]
    wcomb[:, :D] *= 1.0 / N
    wcombt = np.ascontiguousarray(wcomb.T.reshape(3, P, D)).astype(ml_dtypes.bfloat16)
    biasq = (Wq @ b_lin + bq_v).astype(np.float32).reshape(D, 1)
    wk_in = np.ascontiguousarray(Wk).astype(ml_dtypes.bfloat16)

    # headscat[j, 32s + 8s + h] = head-h indicator * 1/sqrt(HD); zeros elsewhere.
    # Column block s (32 wide) is the zero-padded stationary slot for the quad's
    # batch s; within it the batch's 8 head-columns sit at offset 8s.
    headscat = np.zeros((D, P), dtype=np.float32)
    for s in range(QS):
        for h in range(H):
            for j in range(D):
                if j // HD == h:
                    headscat[j, 32 * s + 8 * s + h] = 1.0 / np.sqrt(HD)
    headscat = headscat.astype(ml_dtypes.bfloat16)

    # indmask[r, 8b + h] = 1 if r == b: routes mask row b to its 8 psum rows
    indmask = np.zeros((P, P), dtype=np.float32)
    for b in range(BPC):
        for h in range(H):
            indmask[b, 8 * b + h] = 1.0
    indmask = indmask.astype(ml_dtypes.bfloat16)

    # ind16[8b + h, b] = 1/H: combine folds the head average (1/Z via recip)
    ind16 = np.zeros((P, BPC), dtype=np.float32)
    for b in range(BPC):
        for h in range(H):
            ind16[8 * b + h, b] = 1.0 / H
    ind16 = ind16.astype(ml_dtypes.bfloat16)

    ident32 = np.eye(2 * BPC, dtype=np.float32).astype(ml_dtypes.bfloat16)
    ident128 = np.eye(P, dtype=np.float32).astype(ml_dtypes.bfloat16)

    in_maps = []
    for c in range(NCORES):
        lo = c * BPC
        xs = xbf[lo : lo + BPC]                               # [16, 2048, 128]
        xtc = np.ascontiguousarray(xs.transpose(0, 2, 1))     # [16, 128, 2048]
        xnc = np.ascontiguousarray(xs.reshape(BPC * N, D))
        gi = np.concatenate(
            [
                np.arange(BPC, dtype=np.int32) * N + first_node[lo : lo + BPC, 0],
                np.arange(BPC, dtype=np.int32) * N + current_node[lo : lo + BPC, 0],
            ]
        ).reshape(2 * BPC, 1).astype(np.int32)
        mneg = (mask[lo : lo + BPC].astype(np.float32) * MASKVAL).astype(
            ml_dtypes.bfloat16
        )
        in_maps.append(
            {
                "xt": xtc,
                "xn": xnc,
                "gidx": gi,
                "maskneg": mneg,
                "indmask": indmask,
                "ident32": ident32,
                "ident128": ident128,
                "ind16": ind16,
                "wcombt": wcombt,
                "wk": wk_in,
                "headscat": headscat,
                "biasq": biasq,
            }
        )
    return in_maps


def kernel(**inputs) -> np.ndarray:
    nc = build_nc()
    in_maps = make_in_maps(**inputs)
    res = run_bass_kernel_spmd(nc, in_maps, core_ids=list(range(NCORES)))
    outs = [np.asarray(res.results[c]["out"]) for c in range(NCORES)]
    return np.concatenate(outs, axis=0)


# revision 15
# speedup vs baseline: 1.0086x; 1.0086x over previous
"""Trainium2 Bass kernel for nn_Actor_87497073754359.

Math (per batch b of B=128, x[b] is [N=2048, D=128] f32):
  graph_emb = mean_n x[b];  first/curr = x[b, idx]
  q = Wq @ (W_lin @ concat(graph_emb, first, curr) + b_lin) + bq  -> [H=8, HD=16]
  scores[h, n] = q[h] . (x @ Wk.T)[n, h*16:+16] / 4 ; mask; softmax over n
  out[b] = mean_h softmax

Never materialize k = x@Wk.T. Fold q into Wk:
  t[b][c, h] = sum_j Wk[j, c] * headsel_h(j) * q[b, j] * 0.25
  scores[b][h, n] = sum_c t[b][c, h] * xT[b][c, n]
x streams once as a host-pretransposed bf16 copy, interleaved over two
DMA queues (sync: even tiles, gpsimd: odd tiles) to saturate HBM while
keeping per-quad arrival order.

Layout: all 16 batches' heads share one PSUM tile per n-chunk of 512
(row = 8*b + h -> 128 rows).  Per chunk: one mask matmul, 16 per-batch
score matmuls (zero-padded [128,32] stationaries via PE column tiling),
one Exp, one combine matmul (rmat folds 1/Z and the 1/H head-average).
Row sums for the mean are spread across engines in each quad of
batches: DVE tensor_reduce, ACT accumulate-copy, and a PE
identity-matmul whose psum is collapsed on DVE/ACT — each emitted in
expected-readiness order so no FIFO queue head-blocks.  The last batch
streams as two half-tiles with ACT partial sums and a private mini
q-chain, so only its 4 matmuls + softmax trail the stream.  1/N is
folded into the host-combined Wq@W_lin.

Sharding: pure data parallel over batch (16/core), no collectives.
"""

import numpy as np
import ml_dtypes

import concourse.bass as bass
import concourse.tile as tile
from concourse import bacc, mybir
from concourse.bass_utils import run_bass_kernel_spmd

B, N, D, H = 128, 2048, 128, 8
HD = D // H
NCORES = 8
BPC = B // NCORES          # 16 batches per core
P = 128
CH = 512                   # psum-bank chunk of n
NCH = N // CH              # 4
NQ = 4                     # batch quads per core
QS = BPC // NQ             # 4 batches per quad
LASTB = BPC - 1
MASKVAL = -1000.0          # exp(-1000 + s) == 0.0 exactly in f32

BF16 = mybir.dt.bfloat16
F32 = mybir.dt.float32
I32 = mybir.dt.int32


def build_kernel_body(ctx, tc):
    nc = tc.nc

    # ---- DRAM parameters (per-core shapes) ----
    xt = nc.dram_tensor("xt", [BPC, P, N], BF16, kind="ExternalInput")
    xn = nc.dram_tensor("xn", [BPC * N, D], BF16, kind="ExternalInput")
    gidx = nc.dram_tensor("gidx", [2 * BPC, 1], I32, kind="ExternalInput")
    maskneg = nc.dram_tensor("maskneg", [BPC, N], BF16, kind="ExternalInput")
    indmask = nc.dram_tensor("indmask", [P, P], BF16, kind="ExternalInput")
    ident32_d = nc.dram_tensor("ident32", [2 * BPC, 2 * BPC], BF16, kind="ExternalInput")
    ident128_d = nc.dram_tensor("ident128", [P, P], BF16, kind="ExternalInput")
    ind16 = nc.dram_tensor("ind16", [P, BPC], BF16, kind="ExternalInput")
    wcombt = nc.dram_tensor("wcombt", [3, P, D], BF16, kind="ExternalInput")
    wk = nc.dram_tensor("wk", [D, D], BF16, kind="ExternalInput")
    headscat = nc.dram_tensor("headscat", [D, P], BF16, kind="ExternalInput")
    biasq = nc.dram_tensor("biasq", [D, 1], F32, kind="ExternalInput")
    out = nc.dram_tensor("out", [BPC, N], F32, kind="ExternalOutput")

    consts = ctx.enter_context(tc.tile_pool(name="consts", bufs=1))
    xt_pool = ctx.enter_context(tc.tile_pool(name="xt", bufs=BPC))
    small = ctx.enter_context(tc.tile_pool(name="small", bufs=2))
    mscr_pool = ctx.enter_context(tc.tile_pool(name="mscr", bufs=2))
    w_pool = ctx.enter_context(tc.tile_pool(name="w", bufs=NCH))
    psum_small = ctx.enter_context(tc.tile_pool(name="ps_small", bufs=2, space="PSUM"))
    psum_scores = ctx.enter_context(
        tc.tile_pool(name="ps_scores", bufs=NCH, space="PSUM")
    )
    psum_out = ctx.enter_context(tc.tile_pool(name="ps_out", bufs=2, space="PSUM"))

    # ---- x stream tiles; gather index goes first on the scalar queue so the
    # gpsimd (odd-tile) queue can start streaming almost immediately ----
    xt_tiles = [
        xt_pool.tile([P, N], BF16, tag="xt", name=f"xt{b}") for b in range(BPC)
    ]
    gidx_sb = consts.tile([2 * BPC, 1], I32)
    nc.scalar.dma_start(gidx_sb, gidx[:])
    for b in range(0, BPC, 2):
        nc.sync.dma_start(xt_tiles[b], xt[b])
    grows = consts.tile([2 * BPC, D], BF16)
    nc.gpsimd.indirect_dma_start(
        out=grows[:],
        out_offset=None,
        in_=xn[:],
        in_offset=bass.IndirectOffsetOnAxis(ap=gidx_sb[:, :1], axis=0),
    )
    for b in range(1, LASTB, 2):
        nc.gpsimd.dma_start(xt_tiles[b], xt[b])
    nc.gpsimd.dma_start(xt_tiles[LASTB][:, : N // 2], xt[LASTB, :, : N // 2])
    nc.gpsimd.dma_start(xt_tiles[LASTB][:, N // 2 :], xt[LASTB, :, N // 2 :])

    # ---- constants into SBUF (scalar queue), in dependency-priority order ----
    maskneg_sb = consts.tile([P, N], BF16)
    nc.vector.memset(maskneg_sb, 0.0)
    nc.scalar.dma_start(maskneg_sb[:BPC, :], maskneg[:])
    indmask_sb = consts.tile([P, P], BF16)
    nc.scalar.dma_start(indmask_sb, indmask[:])
    ident32 = consts.tile([2 * BPC, 2 * BPC], BF16)
    nc.scalar.dma_start(ident32, ident32_d[:])
    ident128 = consts.tile([P, P], BF16)
    nc.scalar.dma_start(ident128, ident128_d[:])
    wcombt_sb = consts.tile([P, 3, D], BF16)
    nc.scalar.dma_start(wcombt_sb, wcombt[:].rearrange("p c j -> c p j"))
    wk_sb = consts.tile([D, D], BF16)
    nc.scalar.dma_start(wk_sb, wk[:])
    headscat_sb = consts.tile([D, NQ, 32], BF16)
    nc.scalar.dma_start(headscat_sb[:].rearrange("d q x -> d (q x)"), headscat[:])
    biasq_sb = consts.tile([D, 1], F32)
    nc.scalar.dma_start(biasq_sb, biasq[:])
    ind16_sb = consts.tile([P, BPC], BF16)
    nc.scalar.dma_start(ind16_sb, ind16[:])

    # ---- PE warm-up: dense matmuls so HAM reaches 8/8 early ----
    warm_src = consts.tile([P, CH], BF16)
    nc.vector.memset(warm_src, 1.0)
    for i in range(4):
        pw = psum_small.tile([P, CH], F32, tag="ps", name=f"warm{i}")
        nc.tensor.matmul(
            out=pw[:], lhsT=warm_src[:, :P], rhs=warm_src[:], start=True, stop=True
        )

    # ---- the 4 score psum tiles (one per n-chunk), mask matmul first ----
    score_ps = []
    for ch in range(NCH):
        ps = psum_scores.tile([P, CH], F32, space="PSUM", tag="pscore", name=f"sc{ch}")
        nc.tensor.matmul(
            out=ps[:],
            lhsT=indmask_sb[:],
            rhs=maskneg_sb[:, ch * CH : (ch + 1) * CH],
            start=True,
            stop=False,
            skip_group_check=True,
        )
        score_ps.append(ps)

    # ---- gathered rows -> featsT [128, 32] bf16 (transpose on PE) ----
    psum_f = psum_small.tile([P, 2 * BPC], BF16, space="PSUM", tag="ps")
    nc.tensor.transpose(psum_f[:], grows[:], ident32[:])
    featsT_sb = consts.tile([P, 2 * BPC], BF16)
    nc.vector.tensor_copy(featsT_sb[:], psum_f[:])

    # ---- per-batch means across engines ----
    # sums_f32 col b = row-sum of batch b; col BPC = second partial of LASTB
    sums_f32 = consts.tile([P, BPC + 1], F32)
    sums_bf = consts.tile([P, BPC + 1], BF16)

    def emit_mean_dve(b):
        nc.vector.tensor_reduce(
            out=sums_f32[:, b : b + 1],
            in_=xt_tiles[b][:],
            axis=mybir.AxisListType.X,
            op=mybir.AluOpType.add,
        )

    def emit_mean_act(b, col, lo=0, hi=N):
        scr = mscr_pool.tile([P, N], BF16, tag="mscr", name=f"mscr{b}_{col}")
        nc.scalar.activation(
            out=scr[:, lo:hi],
            in_=xt_tiles[b][:, lo:hi],
            func=mybir.ActivationFunctionType.Copy,
            accum_out=sums_f32[:, col : col + 1],
        )

    def emit_mean_pe_mms(b):
        # identity-stationary matmul: psum[:, j] accumulates x[:, k*512 + j]
        pm = psum_small.tile([P, CH], F32, space="PSUM", tag="ps", name=f"pm{b}")
        for k in range(NCH):
            nc.tensor.matmul(
                out=pm[:],
                lhsT=ident128[:],
                rhs=xt_tiles[b][:, k * CH : (k + 1) * CH],
                start=(k == 0),
                stop=(k == NCH - 1),
            )
        return pm

    def emit_pe_collapse_dve(pm, b):
        nc.vector.tensor_reduce(
            out=sums_f32[:, b : b + 1],
            in_=pm[:],
            axis=mybir.AxisListType.X,
            op=mybir.AluOpType.add,
        )

    def emit_pe_collapse_act(pm, b):
        pescr = mscr_pool.tile([P, CH], BF16, tag="pescr", name=f"pescr{b}")
        nc.scalar.activation(
            out=pescr[:],
            in_=pm[:],
            func=mybir.ActivationFunctionType.Copy,
            accum_out=sums_f32[:, b : b + 1],
        )

    def emit_chain(cols, hs_lo, hs_n, extra_partial=False, name=""):
        """q-chain for contiguous batches [cols] -> statq tile [P, 32*hs_n]."""
        hi = BPC + 1 if extra_partial else cols[-1] + 1
        nc.vector.tensor_copy(sums_bf[:, cols[0] : hi], sums_f32[:, cols[0] : hi])
        psum_q = psum_small.tile(
            [P, len(cols)], F32, space="PSUM", tag="ps", name=f"pq{name}"
        )
        ctx_chunks = [
            sums_bf[:, cols[0] : cols[-1] + 1],
            featsT_sb[:, cols[0] : cols[-1] + 1],
            featsT_sb[:, BPC + cols[0] : BPC + cols[-1] + 1],
        ]
        for pch in range(3):
            nc.tensor.matmul(
                out=psum_q[:],
                lhsT=wcombt_sb[:, pch, :],
                rhs=ctx_chunks[pch],
                start=(pch == 0),
                stop=(pch == 2 and not extra_partial),
                skip_group_check=True,
            )
        if extra_partial:
            # second half-sum of the last batch folds in via one FD=1 matmul
            nc.tensor.matmul(
                out=psum_q[:, len(cols) - 1 :],
                lhsT=wcombt_sb[:, 0, :],
                rhs=sums_bf[:, BPC : BPC + 1],
                start=False,
                stop=True,
                skip_group_check=True,
            )
        qb = small.tile([P, len(cols)], BF16, tag="qb", name=f"qb{name}")
        nc.vector.tensor_scalar(
            out=qb[:],
            in0=psum_q[:],
            scalar1=biasq_sb[:, 0:1],
            scalar2=None,
            op0=mybir.AluOpType.add,
        )
        # qm[j, 32s + x] = headscat[j, hs_lo+s, x] * qb[j, s]
        qm = small.tile([P, hs_n, 32], BF16, tag="qm", name=f"qm{name}")
        nc.vector.tensor_tensor(
            out=qm[:],
            in0=headscat_sb[:, hs_lo : hs_lo + hs_n, :],
            in1=qb[:, :, None].to_broadcast([P, hs_n, 32]),
            op=mybir.AluOpType.mult,
        )
        psum_t = psum_small.tile(
            [P, hs_n * 32], F32, space="PSUM", tag="ps", name=f"pt{name}"
        )
        nc.tensor.matmul(
            out=psum_t[:],
            lhsT=wk_sb[:],
            rhs=qm[:].rearrange("p q x -> p (q x)"),
            start=True,
            stop=True,
        )
        statq = consts.tile([P, hs_n * 32], BF16, name=f"statq{name}")
        nc.vector.tensor_copy(statq[:], psum_t[:])
        return statq

    def emit_scores(q, s, b, statq, stat_s, stop):
        for ch in range(NCH):
            nc.tensor.matmul(
                out=score_ps[ch][32 * q : 32 * q + 32, :],
                lhsT=statq[:, 32 * stat_s : 32 * stat_s + 32],
                rhs=xt_tiles[b][:, ch * CH : (ch + 1) * CH],
                start=False,
                stop=stop,
                skip_group_check=True,
                tile_position=(0, 32 * q),
            )

    for q in range(NQ):
        b0 = q * QS
        last_quad = q == NQ - 1
        # means: s0 -> DVE, s1 -> ACT, s2 -> PE (collapse alternates DVE/ACT),
        # s3 -> DVE on even quads / ACT on odd quads; the very last batch uses
        # two ACT half-sums folded in at the chain.
        emit_mean_dve(b0)
        emit_mean_act(b0 + 1, b0 + 1)
        pm = emit_mean_pe_mms(b0 + 2)
        if q % 2 == 0:
            emit_pe_collapse_dve(pm, b0 + 2)
        else:
            emit_pe_collapse_act(pm, b0 + 2)
        if last_quad:
            emit_mean_act(LASTB, LASTB, 0, N // 2)
            emit_mean_act(LASTB, BPC, N // 2, N)
        elif q % 2 == 0:
            emit_mean_dve(b0 + 3)
        else:
            emit_mean_act(b0 + 3, b0 + 3)

        if not last_quad:
            statq = emit_chain(list(range(b0, b0 + QS)), 0, NQ, name=f"{q}")
            for s in range(QS):
                emit_scores(q, s, b0 + s, statq, s, stop=False)
        else:
            statA = emit_chain([b0, b0 + 1, b0 + 2], 0, 3, name="A")
            for s in range(3):
                emit_scores(q, s, b0 + s, statA, s, stop=False)
            statB = emit_chain([LASTB], 3, 1, extra_partial=True, name="B")
            emit_scores(q, 3, LASTB, statB, 0, stop=True)

    # ---- exp (ACT), Z (DVE), rmat, combine (PE), copy out, DMA ----
    zpart = consts.tile([P, NCH], F32)
    ztot = consts.tile([P, 1], F32)
    recip = consts.tile([P, 1], F32)
    rmat = consts.tile([P, BPC], BF16)
    w_tiles = []
    for ch in range(NCH):
        wt = w_pool.tile([P, CH], BF16, tag="w", name=f"w{ch}")
        nc.scalar.activation(
            out=wt[:],
            in_=score_ps[ch][:],
            func=mybir.ActivationFunctionType.Exp,
        )
        nc.vector.tensor_reduce(
            out=zpart[:, ch : ch + 1],
            in_=wt[:],
            axis=mybir.AxisListType.X,
            op=mybir.AluOpType.add,
        )
        w_tiles.append(wt)
    nc.vector.tensor_reduce(
        out=ztot[:], in_=zpart[:], axis=mybir.AxisListType.X, op=mybir.AluOpType.add
    )
    nc.vector.reciprocal(recip[:], ztot[:])
    nc.vector.tensor_scalar(
        out=rmat[:],
        in0=ind16_sb[:],
        scalar1=recip[:, 0:1],
        scalar2=None,
        op0=mybir.AluOpType.mult,
    )
    out_sb = consts.tile([BPC, N], F32)
    for ch in range(NCH):
        psum_o = psum_out.tile([BPC, CH], F32, space="PSUM", tag="po")
        nc.tensor.matmul(
            out=psum_o[:], lhsT=rmat[:], rhs=w_tiles[ch][:], start=True, stop=True
        )
        cp = nc.scalar.copy if ch % 2 == 0 else nc.vector.tensor_copy
        cp(out_sb[:, ch * CH : (ch + 1) * CH], psum_o[:])
        nc.sync.dma_start(
            out[:, ch * CH : (ch + 1) * CH], out_sb[:, ch * CH : (ch + 1) * CH]
        )


_NC_CACHE = None


def build_nc():
    global _NC_CACHE
    if _NC_CACHE is not None:
        return _NC_CACHE
    from contextlib import ExitStack

    nc = bacc.Bacc("TRN2", target_bir_lowering=False, debug=False)
    with tile.TileContext(nc) as tc:
        with ExitStack() as ctx:
            build_kernel_body(ctx, tc)
    nc.compile()
    _NC_CACHE = nc
    return nc


def make_in_maps(x, first_node, current_node, mask, W_lin, b_lin, Wq, bq, Wk, bk):
    """Host-side sharding/layout prep. Returns list of 8 per-core input dicts."""
    x = np.asarray(x, dtype=np.float32)
    mask = np.asarray(mask)
    first_node = np.asarray(first_node).astype(np.int32)
    current_node = np.asarray(current_node).astype(np.int32)
    W_lin = np.asarray(W_lin, dtype=np.float32)
    b_lin = np.asarray(b_lin, dtype=np.float32)
    Wq = np.asarray(Wq, dtype=np.float32)
    bq_v = np.asarray(bq, dtype=np.float32)
    Wk = np.asarray(Wk, dtype=np.float32)

    xbf = x.astype(ml_dtypes.bfloat16)

    # replicated weights; 1/N for the mean is folded into Wcomb chunk 0
    wcomb = (Wq @ W_lin).astype(np.float32)            # [D, 3D]
    wcomb[:, :D] *= 1.0 / N
    wcombt = np.ascontiguousarray(wcomb.T.reshape(3, P, D)).astype(ml_dtypes.bfloat16)
    biasq = (Wq @ b_lin + bq_v).astype(np.float32).reshape(D, 1)
    wk_in = np.ascontiguousarray(Wk).astype(ml_dtypes.bfloat16)

    # headscat[j, 32s + 8s + h] = head-h indicator * 1/sqrt(HD); zeros elsewhere.
    # Column block s (32 wide) is the zero-padded stationary slot for the quad's
    # batch s; within it the batch's 8 head-columns sit at offset 8s.
    headscat = np.zeros((D, P), dtype=np.float32)
    for s in range(QS):
        for h in range(H):
            for j in range(D):
                if j // HD == h:
                    headscat[j, 32 * s + 8 * s + h] = 1.0 / np.sqrt(HD)
    headscat = headscat.astype(ml_dtypes.bfloat16)

    # indmask[r, 8b + h] = 1 if r == b: routes mask row b to its 8 psum rows
    indmask = np.zeros((P, P), dtype=np.float32)
    for b in range(BPC):
        for h in range(H):
            indmask[b, 8 * b + h] = 1.0
    indmask = indmask.astype(ml_dtypes.bfloat16)

    # ind16[8b + h, b] = 1/H: combine folds the head average (1/Z via recip)
    ind16 = np.zeros((P, BPC), dtype=np.float32)
    for b in range(BPC):
        for h in range(H):
            ind16[8 * b + h, b] = 1.0 / H
    ind16 = ind16.astype(ml_dtypes.bfloat16)

    ident32 = np.eye(2 * BPC, dtype=np.float32).astype(ml_dtypes.bfloat16)
    ident128 = np.eye(P, dtype=np.float32).astype(ml_dtypes.bfloat16)

    in_maps = []
    for c in range(NCORES):
        lo = c * BPC
        xs = xbf[lo : lo + BPC]                               # [16, 2048, 128]
        xtc = np.ascontiguousarray(xs.transpose(0, 2, 1))     # [16, 128, 2048]
        xnc = np.ascontiguousarray(xs.reshape(BPC * N, D))
        gi = np.concatenate(
            [
                np.arange(BPC, dtype=np.int32) * N + first_node[lo : lo + BPC, 0],
                np.arange(BPC, dtype=np.int32) * N + current_node[lo : lo + BPC, 0],
            ]
        ).reshape(2 * BPC, 1).astype(np.int32)
        mneg = (mask[lo : lo + BPC].astype(np.float32) * MASKVAL).astype(
            ml_dtypes.bfloat16
        )
        in_maps.append(
            {
                "xt": xtc,
                "xn": xnc,
                "gidx": gi,
                "maskneg": mneg,
                "indmask": indmask,
                "ident32": ident32,
                "ident128": ident128,
                "ind16": ind16,
                "wcombt": wcombt,
                "wk": wk_in,
                "headscat": headscat,
                "biasq": biasq,
            }
        )
    return in_maps


def kernel(**inputs) -> np.ndarray:
    nc = build_nc()
    in_maps = make_in_maps(**inputs)
    res = run_bass_kernel_spmd(nc, in_maps, core_ids=list(range(NCORES)))
    outs = [np.asarray(res.results[c]["out"]) for c in range(NCORES)]
    return np.concatenate(outs, axis=0)
